# revision 49
# baseline (speedup 1.0000x reference)
"""CP tensor product ('uvu' connection) kernel for Trainium2, SPMD over 8 NeuronCores.

Math per batch element b (reassociation of the reference einsum):
  q   = x2[b] @ w[b].T               (16, 64)
  t1  = A.T @ x1[b]                  (64, 64)
  t3  = B.T @ q                      (64, 64)
  m   = t1 * t3                      (64, 64)  elementwise
  out = C @ m                        (16, 64)

PE cost on this target is proportional to moving columns streamed (independent
of K and M; stationary loads are free), so we pack multiple batch elements per
matmul with block-diagonal stationary tiles:

  q:   stationary blockdiag(x2[b0].T..x2[b3].T) (128=32v*4g, 64=16j*4g) -> 16 cols/b
  t1:  stationary blockdiag(A, A) zero-masked band (128, 128)           -> 32 cols/b
  t3:  stationary blockdiag(B, B) zero-masked band (128, 128)           -> 32 cols/b
  out: stationary blockdiag(C.T,C.T) (128=64r*2e, 32=16c*2e)            -> 32 cols/b

All matmuls are K=128 at PE tile (0,0): t1/t3 stationaries are zero-masked
band variants (only rows [32*band, 32*band+32) nonzero) because the hardware
mishandles PE row tiling when multiple row tiles target one PSUM bank, and
zero rows are free (cost depends only on moving columns).  Two t3 pairs that
share a band are merged into one N=128 matmul via the window permutation
w = (u2%2, p2, u2//2) over batch pairs (UOFW/WOFU below).

All matmuls bf16 moving (1 cyc/row), fp32 PSUM accumulation.  t3 is copied
PSUM->SBUF (Act engine, ~80% / DVE ~20%) so the DVE multiply reads only one
PSUM operand (hardware limit).  Host stages DRAM in device-native layouts
(batch-major permutations + bf16 casts + zero-padded block-diagonal x2) so
every DMA is large and contiguous; next-chunk inputs are prefetched ahead of
the current chunk's output DMAs.

Batch mapping per core (b_local in [0, 4096)):
  b = 512*c + 64*sp + 16*k + 2*u + e   (chunk c<8, super sp<8, quarter k<4,
                                        pair u<8, element-of-pair e<2)
  window w = WOFU[u] orders pairs inside a quarter; 4-b group index
  T = b//4, g = b%4 (g = 2*(u%2)+e).
"""
import numpy as np
from contextlib import ExitStack

import jax
from jax.experimental.shard_map import shard_map
from jax.sharding import Mesh, PartitionSpec, NamedSharding

import concourse.bass as bass
import concourse.bacc as bacc
import concourse.tile as tile
import concourse.mybir as mybir
from concourse._compat import with_exitstack
from concourse.bass2jax import _bass_exec_p, install_neuronx_cc_hook, partition_id_tensor

F32 = mybir.dt.float32
BF16 = mybir.dt.bfloat16
NP_BF16 = mybir.dt.np(mybir.dt.bfloat16)

NCORES = 8
BATCH = 32768
B_LOCAL = BATCH // NCORES
D = 16
CH1 = 64
CH2 = 32
RANK = 64

CHUNK = 512               # batch elems per DMA chunk
NCHUNK = B_LOCAL // CHUNK # 8
NSUPER = 8                # supers (64 b) per chunk


def _emit(ctx: ExitStack, tc: tile.TileContext, outs, ins, nchunk=NCHUNK):
    SPILL = 5   # every 5th t3 PSUM->SBUF copy runs on DVE (Act/DVE balance)
    OSPILL = int(_os.environ.get("K_OSPILL", "0"))
    nc = tc.nc
    (out_d,) = outs
    (x1_d, w_d, x2_d, abd_d, bbd_d, ctbd_d) = ins

    const = ctx.enter_context(tc.tile_pool(name="const", bufs=1))
    # Abd4/Bbd4: four zero-masked band variants (nonzero rows [32*band, +32))
    # so every matmul is K=128 at tile (0,0) -- no PE row tiling, which the
    # hardware mishandles when multiple row tiles target one PSUM bank.
    Abd4 = const.tile([128, 512], BF16)
    Bbd4 = const.tile([128, 512], BF16)
    CTbd = const.tile([128, 32], BF16)

    def _load_consts():
        nc.sync.dma_start(Abd4[:].rearrange("p (b f) -> p b f", f=128),
                          abd_d[:, :, :].rearrange("b p f -> p b f"))
        nc.sync.dma_start(Bbd4[:].rearrange("p (b f) -> p b f", f=128),
                          bbd_d[:, :, :].rearrange("b p f -> p b f"))
        nc.sync.dma_start(CTbd[:], ctbd_d[:, :])

    x1p = ctx.enter_context(tc.tile_pool(name="x1", bufs=2))
    wp = ctx.enter_context(tc.tile_pool(name="w", bufs=2))
    x2p = ctx.enter_context(tc.tile_pool(name="x2", bufs=2))
    qsbp = ctx.enter_context(tc.tile_pool(name="qsb", bufs=3))
    t3sbp = ctx.enter_context(tc.tile_pool(name="t3sb", bufs=8))
    msbp = ctx.enter_context(tc.tile_pool(name="msb", bufs=8))
    osbp = ctx.enter_context(tc.tile_pool(name="osb", bufs=2))
    pq = ctx.enter_context(tc.tile_pool(name="pq", bufs=2, space="PSUM"))
    pt1 = ctx.enter_context(tc.tile_pool(name="pt1", bufs=2, space="PSUM"))
    pt3 = ctx.enter_context(tc.tile_pool(name="pt3", bufs=2, space="PSUM"))
    po = ctx.enter_context(tc.tile_pool(name="po", bufs=2, space="PSUM"))

    def _load_chunk(c):
        # q inputs (x2, w) first and split so the first q matmuls start early
        x2_t = x2p.tile([128, 8192], BF16)    # p=32g+v, f=64*tl+16*g'+j (host-padded bd)
        w_t = wp.tile([128, 8192], BF16)      # p=32g+v, f=64*tl+o   (tl = T%128)
        x1_t = x1p.tile([128, 4096], BF16)    # p=32*(sp%4)+16e+i, f=2048*(sp//4)+512k+64w+o
        if c == 0:
            # fine-grained first-chunk loads so super 0 starts ~4us in
            for h in range(8):
                fh = 1024 * h
                nc.sync.dma_start(x2_t[:, fh:fh + 1024],
                                  x2_d[c, :, fh:fh + 1024])
                nc.sync.dma_start(
                    w_t[:, fh:fh + 1024].rearrange("p (t o) -> p t o", o=64),
                    w_d[:, 128 * c + 16 * h:128 * c + 16 * (h + 1), :])
                if h < 4:
                    nc.sync.dma_start(x1_t[:, 1024 * h:1024 * (h + 1)],
                                      x1_d[c, :, 1024 * h:1024 * (h + 1)])
                if h == 0:
                    _load_consts()
        else:
            for h in range(2):
                fh = 4096 * h
                nc.sync.dma_start(x2_t[:, fh:fh + 4096],
                                  x2_d[c, :, fh:fh + 4096])
                nc.sync.dma_start(
                    w_t[:, fh:fh + 4096].rearrange("p (t o) -> p t o", o=64),
                    w_d[:, 128 * c + 64 * h:128 * c + 64 * (h + 1), :])
            nc.sync.dma_start(x1_t[:], x1_d[c])
        return x1_t, w_t, x2_t

    qi = 0  # global quarter counter (for Act/DVE copy balancing)
    nxt = _load_chunk(0)
    for c in range(nchunk):
        x1_t, w_t, x2_t = nxt
        if c + 1 < nchunk:
            # prefetch next chunk's inputs ahead of this chunk's out DMAs
            # so the SP queue never parks them behind end-of-chunk work
            nxt = _load_chunk(c + 1)
        o_sb = osbp.tile([128, 4096], F32)    # p=32k+16e+co, f=512sp+64u+o

        for sp in range(NSUPER):
            # ---- q: blockdiag(x2T) stationary, 4 b per matmul ----
            q_ps = pq.tile([128, 512], F32)   # p=64*(ts%2)+16g'+j, f=64*(ts//2)+o
            for ts in range(16):
                tl = 16 * sp + ts
                nc.tensor.matmul(
                    q_ps[64 * (ts % 2):64 * (ts % 2) + 64, 64 * (ts // 2):64 * (ts // 2) + 64],
                    x2_t[:, 64 * tl:64 * tl + 64],
                    w_t[:, 64 * tl:64 * tl + 64],
                    tile_position=(0, 64 * (ts % 2)),
                )
            q_sb = qsbp.tile([128, 512], BF16)
            nc.scalar.copy(q_sb[:], q_ps[:])

            o_ps = po.tile([128, 512], F32)   # p=32k+16e+co, f=64u+o
            for k in range(4):
                # ---- t1: blockdiag(A,A) band variant, 16 b in one matmul ----
                t1_ps = pt1.tile([128, 512], F32)  # p=64e+r, f=64u+o
                nc.tensor.matmul(
                    t1_ps[:],
                    Abd4[:, 128 * (sp % 4):128 * (sp % 4 + 1)],
                    x1_t[:, 2048 * (sp // 4) + 512 * k:
                         2048 * (sp // 4) + 512 * (k + 1)],
                    tile_position=(0, 0),
                )
                # ---- t3: blockdiag(B,B) band variants, 4 b per matmul ----
                # pairs sharing a stationary band are merged (N=128); the
                # window permutation w = (u2%2, p2, u2//2) makes their
                # outputs adjacent (host stages x1/out in the same order).
                t3_ps = pt3.tile([128, 512], F32)  # p=64e+r, f=64w+o
                for band in range(4):              # band = 2*(u2%2) + p2
                    nc.tensor.matmul(
                        t3_ps[:, 128 * band:128 * (band + 1)],
                        Bbd4[:, 128 * band:128 * (band + 1)],
                        q_sb[:, 128 * k:128 * (k + 1)],
                        tile_position=(0, 0),
                    )
                # ---- t3 -> SBUF (one-PSUM-operand rule), then m = t1*t3 ----
                t3_sb = t3sbp.tile([128, 512], F32)
                if qi % SPILL == SPILL - 1:
                    nc.vector.tensor_copy(t3_sb[:], t3_ps[:])
                else:
                    nc.scalar.copy(t3_sb[:], t3_ps[:])
                qi += 1
                m_t = msbp.tile([128, 512], BF16)
                nc.vector.tensor_mul(m_t[:], t1_ps[:], t3_sb[:])
                # ---- out: blockdiag(C.T,C.T), 16 b in one matmul ----
                nc.tensor.matmul(
                    o_ps[32 * k:32 * (k + 1), :],
                    CTbd[:],
                    m_t[:],
                    tile_position=(0, 32 * k),
                )
            nc.scalar.copy(o_sb[:, 512 * sp:512 * (sp + 1)], o_ps[:])
            if c == nchunk - 1:
                nc.sync.dma_start(out_d[c, :, 512 * sp:512 * (sp + 1)],
                                  o_sb[:, 512 * sp:512 * (sp + 1)])
            elif sp % 2 == 1:
                j = sp // 2
                nc.sync.dma_start(out_d[c, :, 1024 * j:1024 * (j + 1)],
                                  o_sb[:, 1024 * j:1024 * (j + 1)])


@with_exitstack
def _cp_kernel(ctx, tc, outs, ins, nchunk=NCHUNK):
    _emit(ctx, tc, outs, ins, nchunk)


def build_nc(b_local: int = B_LOCAL, nchunk: int = NCHUNK):
    assert b_local == B_LOCAL
    nc = bacc.Bacc("TRN2", target_bir_lowering=False, debug=False)
    x1_d = nc.dram_tensor("x1", [nchunk, 128, 4096], BF16, kind="ExternalInput").ap()
    w_d = nc.dram_tensor("w", [128, nchunk * 128, 64], BF16, kind="ExternalInput").ap()
    x2_d = nc.dram_tensor("x2", [nchunk, 128, 8192], BF16, kind="ExternalInput").ap()
    abd_d = nc.dram_tensor("abd", [4, 128, 128], BF16, kind="ExternalInput").ap()
    bbd_d = nc.dram_tensor("bbd", [4, 128, 128], BF16, kind="ExternalInput").ap()
    ctbd_d = nc.dram_tensor("ctbd", [128, 32], BF16, kind="ExternalInput").ap()
    out_d = nc.dram_tensor("out", [nchunk, 128, 4096], F32, kind="ExternalOutput").ap()
    with tile.TileContext(nc, trace_sim=False) as tc:
        _cp_kernel(tc, [out_d], [x1_d, w_d, x2_d, abd_d, bbd_d, ctbd_d], nchunk)
    nc.compile()
    return nc


# ---------------- host-side staging ----------------

UOFW = [0, 4, 1, 5, 2, 6, 3, 7]   # window w holds batch pair u = UOFW[w]
WOFU = [0, 2, 4, 6, 1, 3, 5, 7]   # inverse


def _prep_x1(x1c: np.ndarray) -> np.ndarray:
    """(4096, 16, 64) fp32 -> bf16 (8, 128, 4096): p=32*spl+16e+i,
    f=2048*sph+512k+64w+o with b = 512c + 256*sph + 64*spl + 16k + 2*UOFW[w] + e."""
    a = x1c.reshape(NCHUNK, 2, 4, 4, 8, 2, D, CH1)          # c sph spl k u e i o
    a = a[:, :, :, :, UOFW]                                  # u-axis -> w order
    a = a.transpose(0, 2, 5, 6, 1, 3, 4, 7)                  # c spl e i sph k w o
    return np.ascontiguousarray(a.reshape(NCHUNK, 128, 4096).astype(NP_BF16))


def _prep_w(wc: np.ndarray) -> np.ndarray:
    """(4096, 64, 32) fp32 -> bf16 (128, 1024, 64): [32g+v, T, o] = w[4T+g, o, v]."""
    a = wc.reshape(B_LOCAL // 4, 4, CH1, CH2)                # T g o v
    a = a.transpose(1, 3, 0, 2)                              # g v T o
    return np.ascontiguousarray(a.reshape(128, B_LOCAL // 4, CH1).astype(NP_BF16))


def _prep_x2(x2c: np.ndarray) -> np.ndarray:
    """(4096, 16, 32) fp32 -> bf16 (8, 128, 8192) zero-padded block-diagonal:
    [c, 32g+v, 64tl + 16g' + j] = x2[4*(128c+tl)+g, j, v] if g'==g else 0."""
    a = x2c.reshape(NCHUNK, 128, 4, D, CH2)                  # c tl g j v
    a = a.transpose(0, 2, 4, 1, 3)                           # c g v tl j
    out = np.zeros((NCHUNK, 4, CH2, 128, 4, D), NP_BF16)     # c g_p v tl g' j
    for g in range(4):
        out[:, g, :, :, g] = a[:, g]
    return np.ascontiguousarray(out.reshape(NCHUNK, 128, 8192))


def _prep_consts(A: np.ndarray, B: np.ndarray, C: np.ndarray):
    abd = np.zeros((4, 128, 128), np.float32)
    bbd = np.zeros((4, 128, 128), np.float32)
    ctbd = np.zeros((128, 32), np.float32)
    for band in range(4):
        for e in range(2):
            r0 = 32 * band + 16 * e
            abd[band, r0:r0 + 16, 64 * e:64 * e + 64] = A
            bbd[band, r0:r0 + 16, 64 * e:64 * e + 64] = B
    for e in range(2):
        ctbd[64 * e:64 * e + 64, 16 * e:16 * e + 16] = C.T
    return (np.ascontiguousarray(abd.astype(NP_BF16)),
            np.ascontiguousarray(bbd.astype(NP_BF16)),
            np.ascontiguousarray(ctbd.astype(NP_BF16)))


def _decode_out(oc: np.ndarray) -> np.ndarray:
    """(8, 128, 4096) fp32 -> (4096, 16, 64)."""
    a = oc.reshape(NCHUNK, 4, 2, 16, 8, 8, 64)               # c k e co sp w o
    a = a[:, :, :, :, :, WOFU]                               # w-axis -> u order
    a = a.transpose(0, 4, 1, 5, 2, 3, 6)                     # c sp k u e co o
    return np.ascontiguousarray(a.reshape(B_LOCAL, D, CH1))


def prepare_in_maps(x1, x2, w, A, B, C):
    x1 = np.asarray(x1, dtype=np.float32)
    x2 = np.asarray(x2, dtype=np.float32)
    w = np.asarray(w, dtype=np.float32)
    abd, bbd, ctbd = _prep_consts(np.asarray(A, np.float32),
                                  np.asarray(B, np.float32),
                                  np.asarray(C, np.float32))
    in_maps = []
    for core in range(NCORES):
        sl = slice(core * B_LOCAL, (core + 1) * B_LOCAL)
        in_maps.append({
            "x1": _prep_x1(x1[sl]),
            "w": _prep_w(w[sl]),
            "x2": _prep_x2(x2[sl]),
            "abd": abd, "bbd": bbd, "ctbd": ctbd,
        })
    return in_maps


class _SpmdRunner:
    """Persistent jitted SPMD executor over the 8 NeuronCores."""

    def __init__(self, nc, n_cores=NCORES):
        install_neuronx_cc_hook()
        self.nc = nc
        self.n_cores = n_cores
        pid_name = nc.partition_id_tensor.name if nc.partition_id_tensor else None

        in_names, out_names, out_avals, zero_outs = [], [], [], []
        for alloc in nc.m.functions[0].allocations:
            if not isinstance(alloc, mybir.MemoryLocationSet):
                continue
            name = alloc.memorylocations[0].name
            if alloc.kind == "ExternalInput":
                if name != pid_name:
                    in_names.append(name)
            elif alloc.kind == "ExternalOutput":
                out_names.append(name)
                shape = tuple(alloc.tensor_shape)
                dtype = mybir.dt.np(alloc.dtype)
                out_avals.append(jax.core.ShapedArray(shape, dtype))
                zero_outs.append(np.zeros(shape, dtype))
        self.in_names, self.out_names = in_names, out_names
        self.out_avals, self.zero_outs = out_avals, zero_outs
        n_params = len(in_names)
        all_names = tuple(in_names + out_names + ([pid_name] if pid_name else []))

        def _body(*args):
            operands = list(args)
            if pid_name is not None:
                operands.append(partition_id_tensor())
            outs = _bass_exec_p.bind(
                *operands,
                out_avals=tuple(out_avals),
                in_names=all_names,
                out_names=tuple(out_names),
                lowering_input_output_aliases=(),
                sim_require_finite=True,
                sim_require_nnan=True,
                nc=nc,
            )
            return tuple(outs)

        devices = jax.devices()[:n_cores]
        self.mesh = Mesh(np.asarray(devices), ("core",))
        self.sharding = NamedSharding(self.mesh, PartitionSpec("core"))
        n_out = len(out_names)
        donate = tuple(range(n_params, n_params + n_out))
        self.jitted = jax.jit(
            shard_map(_body, mesh=self.mesh,
                      in_specs=(PartitionSpec("core"),) * (n_params + n_out),
                      out_specs=(PartitionSpec("core"),) * n_out,
                      check_rep=False),
            donate_argnums=donate, keep_unused=True,
        )

    def stage_inputs(self, in_maps):
        per_core = [[np.asarray(m[name]) for name in self.in_names] for m in in_maps]
        concat = [np.concatenate([per_core[c][i] for c in range(self.n_cores)], axis=0)
                  for i in range(len(self.in_names))]
        return [jax.device_put(a, self.sharding) for a in concat]

    def stage_zeros(self):
        zs = [np.zeros((self.n_cores * z.shape[0], *z.shape[1:]), z.dtype)
              for z in self.zero_outs]
        return [jax.device_put(z, self.sharding) for z in zs]

    def run(self, dev_inputs, dev_zeros=None):
        if dev_zeros is None:
            dev_zeros = self.stage_zeros()
        outs = self.jitted(*dev_inputs, *dev_zeros)
        jax.block_until_ready(outs)
        return outs

    def unshard_out(self, outs):
        i = self.out_names.index("out")
        a = np.asarray(outs[i])  # (NCORES*NCHUNK, 128, 4096)
        a = a.reshape(NCORES, NCHUNK, 128, 4096)
        return np.concatenate([_decode_out(a[c]) for c in range(NCORES)], axis=0)


_RUNNER = None


def _get_runner():
    global _RUNNER
    if _RUNNER is None:
        nc = build_nc(B_LOCAL)
        _RUNNER = _SpmdRunner(nc, NCORES)
    return _RUNNER


def kernel(x1, x2, w, A, B, C):
    """Full-input entry point. Shards batch across 8 NeuronCores, runs the
    Bass kernel, gathers the full output (32768, 16, 64) float32."""
    runner = _get_runner()
    in_maps = prepare_in_maps(x1, x2, w, A, B, C)
    dev_in = runner.stage_inputs(in_maps)
    outs = runner.run(dev_in)
    return runner.unshard_out(outs)


# revision 54
# speedup vs baseline: 1.0024x; 1.0024x over previous
"""CP tensor product ('uvu' connection) kernel for Trainium2, SPMD over 8 NeuronCores.

Math per batch element b (reassociation of the reference einsum):
  q   = x2[b] @ w[b].T               (16, 64)
  t1  = A.T @ x1[b]                  (64, 64)
  t3  = B.T @ q                      (64, 64)
  m   = t1 * t3                      (64, 64)  elementwise
  out = C @ m                        (16, 64)

PE cost on this target is proportional to moving columns streamed (independent
of K and M; stationary loads are free), so we pack multiple batch elements per
matmul with block-diagonal stationary tiles:

  q:   stationary blockdiag(x2[b0].T..x2[b3].T) (128=32v*4g, 64=16j*4g) -> 16 cols/b
  t1:  stationary blockdiag(A, A) zero-masked band (128, 128)           -> 32 cols/b
  t3:  stationary blockdiag(B, B) zero-masked band (128, 128)           -> 32 cols/b
  out: stationary blockdiag(C.T,C.T) (128=64r*2e, 32=16c*2e)            -> 32 cols/b

All matmuls are K=128 at PE tile (0,0): t1/t3 stationaries are zero-masked
band variants (only rows [32*band, 32*band+32) nonzero) because the hardware
mishandles PE row tiling when multiple row tiles target one PSUM bank, and
zero rows are free (cost depends only on moving columns).  Two t3 pairs that
share a band are merged into one N=128 matmul via the window permutation
w = (u2%2, p2, u2//2) over batch pairs (UOFW/WOFU below).

All matmuls bf16 moving (1 cyc/row), fp32 PSUM accumulation.  t3 is copied
PSUM->SBUF (Act engine, ~80% / DVE ~20%) so the DVE multiply reads only one
PSUM operand (hardware limit).  Host stages DRAM in device-native layouts
(batch-major permutations + bf16 casts + zero-padded block-diagonal x2) so
every DMA is large and contiguous; next-chunk inputs are prefetched ahead of
the current chunk's output DMAs.

Batch mapping per core (b_local in [0, 4096)):
  b = 512*c + 64*sp + 16*k + 2*u + e   (chunk c<8, super sp<8, quarter k<4,
                                        pair u<8, element-of-pair e<2)
  window w = WOFU[u] orders pairs inside a quarter; 4-b group index
  T = b//4, g = b%4 (g = 2*(u%2)+e).
"""
import numpy as np
from contextlib import ExitStack

import jax
from jax.experimental.shard_map import shard_map
from jax.sharding import Mesh, PartitionSpec, NamedSharding

import concourse.bass as bass
import concourse.bacc as bacc
import concourse.tile as tile
import concourse.mybir as mybir
from concourse._compat import with_exitstack
from concourse.bass2jax import _bass_exec_p, install_neuronx_cc_hook, partition_id_tensor

F32 = mybir.dt.float32
BF16 = mybir.dt.bfloat16
NP_BF16 = mybir.dt.np(mybir.dt.bfloat16)

NCORES = 8
BATCH = 32768
B_LOCAL = BATCH // NCORES
D = 16
CH1 = 64
CH2 = 32
RANK = 64

CHUNK = 512               # batch elems per DMA chunk
NCHUNK = B_LOCAL // CHUNK # 8
NSUPER = 8                # supers (64 b) per chunk


def _emit(ctx: ExitStack, tc: tile.TileContext, outs, ins, nchunk=NCHUNK):
    SPILL = 5   # every 5th t3 PSUM->SBUF copy runs on DVE (Act/DVE balance)
    OSPILL = int(_os.environ.get("K_OSPILL", "0"))
    nc = tc.nc
    (out_d,) = outs
    (x1_d, w_d, x2_d, abd_d, bbd_d, ctbd_d) = ins

    const = ctx.enter_context(tc.tile_pool(name="const", bufs=1))
    # Abd4/Bbd4: four zero-masked band variants (nonzero rows [32*band, +32))
    # so every matmul is K=128 at tile (0,0) -- no PE row tiling, which the
    # hardware mishandles when multiple row tiles target one PSUM bank.
    Abd4 = const.tile([128, 512], BF16)
    Bbd4 = const.tile([128, 512], BF16)
    CTbd = const.tile([128, 32], BF16)

    def _load_consts():
        nc.sync.dma_start(Abd4[:].rearrange("p (b f) -> p b f", f=128),
                          abd_d[:, :, :].rearrange("b p f -> p b f"))
        nc.sync.dma_start(Bbd4[:].rearrange("p (b f) -> p b f", f=128),
                          bbd_d[:, :, :].rearrange("b p f -> p b f"))
        nc.sync.dma_start(CTbd[:], ctbd_d[:, :])

    x1p = ctx.enter_context(tc.tile_pool(name="x1", bufs=2))
    wp = ctx.enter_context(tc.tile_pool(name="w", bufs=2))
    x2p = ctx.enter_context(tc.tile_pool(name="x2", bufs=2))
    qsbp = ctx.enter_context(tc.tile_pool(name="qsb", bufs=3))
    t3sbp = ctx.enter_context(tc.tile_pool(name="t3sb", bufs=8))
    msbp = ctx.enter_context(tc.tile_pool(name="msb", bufs=8))
    osbp = ctx.enter_context(tc.tile_pool(name="osb", bufs=2))
    pq = ctx.enter_context(tc.tile_pool(name="pq", bufs=2, space="PSUM"))
    pt1 = ctx.enter_context(tc.tile_pool(name="pt1", bufs=2, space="PSUM"))
    pt3 = ctx.enter_context(tc.tile_pool(name="pt3", bufs=2, space="PSUM"))
    po = ctx.enter_context(tc.tile_pool(name="po", bufs=2, space="PSUM"))

    def _load_chunk(c):
        # q inputs (x2, w) first and split so the first q matmuls start early
        x2_t = x2p.tile([128, 8192], BF16)    # p=32g+v, f=64*tl+16*g'+j (host-padded bd)
        w_t = wp.tile([128, 8192], BF16)      # p=32g+v, f=64*tl+o   (tl = T%128)
        x1_t = x1p.tile([128, 4096], BF16)    # p=32*(sp%4)+16e+i, f=2048*(sp//4)+512k+64w+o
        if c == 0:
            # fine-grained first-chunk loads so super 0 starts ~4us in
            for h in range(8):
                fh = 1024 * h
                nc.sync.dma_start(x2_t[:, fh:fh + 1024],
                                  x2_d[c, :, fh:fh + 1024])
                nc.sync.dma_start(
                    w_t[:, fh:fh + 1024].rearrange("p (t o) -> p t o", o=64),
                    w_d[:, 128 * c + 16 * h:128 * c + 16 * (h + 1), :])
                if h < 4:
                    nc.sync.dma_start(x1_t[:, 1024 * h:1024 * (h + 1)],
                                      x1_d[c, :, 1024 * h:1024 * (h + 1)])
                if h == 0:
                    _load_consts()
        else:
            for h in range(2):
                fh = 4096 * h
                nc.sync.dma_start(x2_t[:, fh:fh + 4096],
                                  x2_d[c, :, fh:fh + 4096])
                nc.sync.dma_start(
                    w_t[:, fh:fh + 4096].rearrange("p (t o) -> p t o", o=64),
                    w_d[:, 128 * c + 64 * h:128 * c + 64 * (h + 1), :])
            nc.sync.dma_start(x1_t[:], x1_d[c])
        return x1_t, w_t, x2_t

    qi = 0  # global quarter counter (for Act/DVE copy balancing)
    nxt = _load_chunk(0)
    for c in range(nchunk):
        x1_t, w_t, x2_t = nxt
        if c + 1 < nchunk:
            # prefetch next chunk's inputs ahead of this chunk's out DMAs
            # so the SP queue never parks them behind end-of-chunk work
            nxt = _load_chunk(c + 1)
        o_sb = osbp.tile([128, 4096], F32)    # p=32k+16e+co, f=512sp+64u+o

        for sp in range(NSUPER):
            # ---- q: blockdiag(x2T) stationary, 4 b per matmul ----
            q_ps = pq.tile([128, 512], F32)   # p=64*(ts%2)+16g'+j, f=64*(ts//2)+o
            for ts in range(16):
                tl = 16 * sp + ts
                nc.tensor.matmul(
                    q_ps[64 * (ts % 2):64 * (ts % 2) + 64, 64 * (ts // 2):64 * (ts // 2) + 64],
                    x2_t[:, 64 * tl:64 * tl + 64],
                    w_t[:, 64 * tl:64 * tl + 64],
                    tile_position=(0, 64 * (ts % 2)),
                )
            q_sb = qsbp.tile([128, 512], BF16)
            nc.scalar.copy(q_sb[:], q_ps[:])

            o_ps = po.tile([128, 512], F32)   # p=32k+16e+co, f=64u+o
            for k in range(4):
                # ---- t1: blockdiag(A,A) band variant, 16 b in one matmul ----
                t1_ps = pt1.tile([128, 512], F32)  # p=64e+r, f=64u+o
                nc.tensor.matmul(
                    t1_ps[:],
                    Abd4[:, 128 * (sp % 4):128 * (sp % 4 + 1)],
                    x1_t[:, 2048 * (sp // 4) + 512 * k:
                         2048 * (sp // 4) + 512 * (k + 1)],
                    tile_position=(0, 0),
                )
                # ---- t3: blockdiag(B,B) band variants, 4 b per matmul ----
                # pairs sharing a stationary band are merged (N=128); the
                # window permutation w = (u2%2, p2, u2//2) makes their
                # outputs adjacent (host stages x1/out in the same order).
                t3_ps = pt3.tile([128, 512], F32)  # p=64e+r, f=64w+o
                for band in range(4):              # band = 2*(u2%2) + p2
                    nc.tensor.matmul(
                        t3_ps[:, 128 * band:128 * (band + 1)],
                        Bbd4[:, 128 * band:128 * (band + 1)],
                        q_sb[:, 128 * k:128 * (k + 1)],
                        tile_position=(0, 0),
                    )
                # ---- t3 -> SBUF (one-PSUM-operand rule), then m = t1*t3 ----
                t3_sb = t3sbp.tile([128, 512], F32)
                if (qi % SPILL == SPILL - 1 and qi > 4) or qi in (0, 1, 2):
                    nc.vector.tensor_copy(t3_sb[:], t3_ps[:])
                else:
                    nc.scalar.copy(t3_sb[:], t3_ps[:])
                qi += 1
                m_t = msbp.tile([128, 512], BF16)
                nc.vector.tensor_mul(m_t[:], t1_ps[:], t3_sb[:])
                # ---- out: blockdiag(C.T,C.T), 16 b in one matmul ----
                nc.tensor.matmul(
                    o_ps[32 * k:32 * (k + 1), :],
                    CTbd[:],
                    m_t[:],
                    tile_position=(0, 32 * k),
                )
            nc.scalar.copy(o_sb[:, 512 * sp:512 * (sp + 1)], o_ps[:])
            if c == nchunk - 1:
                nc.sync.dma_start(out_d[c, :, 512 * sp:512 * (sp + 1)],
                                  o_sb[:, 512 * sp:512 * (sp + 1)])
            elif sp % 2 == 1:
                j = sp // 2
                nc.sync.dma_start(out_d[c, :, 1024 * j:1024 * (j + 1)],
                                  o_sb[:, 1024 * j:1024 * (j + 1)])


@with_exitstack
def _cp_kernel(ctx, tc, outs, ins, nchunk=NCHUNK):
    _emit(ctx, tc, outs, ins, nchunk)


def build_nc(b_local: int = B_LOCAL, nchunk: int = NCHUNK):
    assert b_local == B_LOCAL
    nc = bacc.Bacc("TRN2", target_bir_lowering=False, debug=False)
    x1_d = nc.dram_tensor("x1", [nchunk, 128, 4096], BF16, kind="ExternalInput").ap()
    w_d = nc.dram_tensor("w", [128, nchunk * 128, 64], BF16, kind="ExternalInput").ap()
    x2_d = nc.dram_tensor("x2", [nchunk, 128, 8192], BF16, kind="ExternalInput").ap()
    abd_d = nc.dram_tensor("abd", [4, 128, 128], BF16, kind="ExternalInput").ap()
    bbd_d = nc.dram_tensor("bbd", [4, 128, 128], BF16, kind="ExternalInput").ap()
    ctbd_d = nc.dram_tensor("ctbd", [128, 32], BF16, kind="ExternalInput").ap()
    out_d = nc.dram_tensor("out", [nchunk, 128, 4096], F32, kind="ExternalOutput").ap()
    with tile.TileContext(nc, trace_sim=False) as tc:
        _cp_kernel(tc, [out_d], [x1_d, w_d, x2_d, abd_d, bbd_d, ctbd_d], nchunk)
    nc.compile()
    return nc


# ---------------- host-side staging ----------------

UOFW = [0, 4, 1, 5, 2, 6, 3, 7]   # window w holds batch pair u = UOFW[w]
WOFU = [0, 2, 4, 6, 1, 3, 5, 7]   # inverse


def _prep_x1(x1c: np.ndarray) -> np.ndarray:
    """(4096, 16, 64) fp32 -> bf16 (8, 128, 4096): p=32*spl+16e+i,
    f=2048*sph+512k+64w+o with b = 512c + 256*sph + 64*spl + 16k + 2*UOFW[w] + e."""
    a = x1c.reshape(NCHUNK, 2, 4, 4, 8, 2, D, CH1)          # c sph spl k u e i o
    a = a[:, :, :, :, UOFW]                                  # u-axis -> w order
    a = a.transpose(0, 2, 5, 6, 1, 3, 4, 7)                  # c spl e i sph k w o
    return np.ascontiguousarray(a.reshape(NCHUNK, 128, 4096).astype(NP_BF16))


def _prep_w(wc: np.ndarray) -> np.ndarray:
    """(4096, 64, 32) fp32 -> bf16 (128, 1024, 64): [32g+v, T, o] = w[4T+g, o, v]."""
    a = wc.reshape(B_LOCAL // 4, 4, CH1, CH2)                # T g o v
    a = a.transpose(1, 3, 0, 2)                              # g v T o
    return np.ascontiguousarray(a.reshape(128, B_LOCAL // 4, CH1).astype(NP_BF16))


def _prep_x2(x2c: np.ndarray) -> np.ndarray:
    """(4096, 16, 32) fp32 -> bf16 (8, 128, 8192) zero-padded block-diagonal:
    [c, 32g+v, 64tl + 16g' + j] = x2[4*(128c+tl)+g, j, v] if g'==g else 0."""
    a = x2c.reshape(NCHUNK, 128, 4, D, CH2)                  # c tl g j v
    a = a.transpose(0, 2, 4, 1, 3)                           # c g v tl j
    out = np.zeros((NCHUNK, 4, CH2, 128, 4, D), NP_BF16)     # c g_p v tl g' j
    for g in range(4):
        out[:, g, :, :, g] = a[:, g]
    return np.ascontiguousarray(out.reshape(NCHUNK, 128, 8192))


def _prep_consts(A: np.ndarray, B: np.ndarray, C: np.ndarray):
    abd = np.zeros((4, 128, 128), np.float32)
    bbd = np.zeros((4, 128, 128), np.float32)
    ctbd = np.zeros((128, 32), np.float32)
    for band in range(4):
        for e in range(2):
            r0 = 32 * band + 16 * e
            abd[band, r0:r0 + 16, 64 * e:64 * e + 64] = A
            bbd[band, r0:r0 + 16, 64 * e:64 * e + 64] = B
    for e in range(2):
        ctbd[64 * e:64 * e + 64, 16 * e:16 * e + 16] = C.T
    return (np.ascontiguousarray(abd.astype(NP_BF16)),
            np.ascontiguousarray(bbd.astype(NP_BF16)),
            np.ascontiguousarray(ctbd.astype(NP_BF16)))


def _decode_out(oc: np.ndarray) -> np.ndarray:
    """(8, 128, 4096) fp32 -> (4096, 16, 64)."""
    a = oc.reshape(NCHUNK, 4, 2, 16, 8, 8, 64)               # c k e co sp w o
    a = a[:, :, :, :, :, WOFU]                               # w-axis -> u order
    a = a.transpose(0, 4, 1, 5, 2, 3, 6)                     # c sp k u e co o
    return np.ascontiguousarray(a.reshape(B_LOCAL, D, CH1))


def prepare_in_maps(x1, x2, w, A, B, C):
    x1 = np.asarray(x1, dtype=np.float32)
    x2 = np.asarray(x2, dtype=np.float32)
    w = np.asarray(w, dtype=np.float32)
    abd, bbd, ctbd = _prep_consts(np.asarray(A, np.float32),
                                  np.asarray(B, np.float32),
                                  np.asarray(C, np.float32))
    in_maps = []
    for core in range(NCORES):
        sl = slice(core * B_LOCAL, (core + 1) * B_LOCAL)
        in_maps.append({
            "x1": _prep_x1(x1[sl]),
            "w": _prep_w(w[sl]),
            "x2": _prep_x2(x2[sl]),
            "abd": abd, "bbd": bbd, "ctbd": ctbd,
        })
    return in_maps


class _SpmdRunner:
    """Persistent jitted SPMD executor over the 8 NeuronCores."""

    def __init__(self, nc, n_cores=NCORES):
        install_neuronx_cc_hook()
        self.nc = nc
        self.n_cores = n_cores
        pid_name = nc.partition_id_tensor.name if nc.partition_id_tensor else None

        in_names, out_names, out_avals, zero_outs = [], [], [], []
        for alloc in nc.m.functions[0].allocations:
            if not isinstance(alloc, mybir.MemoryLocationSet):
                continue
            name = alloc.memorylocations[0].name
            if alloc.kind == "ExternalInput":
                if name != pid_name:
                    in_names.append(name)
            elif alloc.kind == "ExternalOutput":
                out_names.append(name)
                shape = tuple(alloc.tensor_shape)
                dtype = mybir.dt.np(alloc.dtype)
                out_avals.append(jax.core.ShapedArray(shape, dtype))
                zero_outs.append(np.zeros(shape, dtype))
        self.in_names, self.out_names = in_names, out_names
        self.out_avals, self.zero_outs = out_avals, zero_outs
        n_params = len(in_names)
        all_names = tuple(in_names + out_names + ([pid_name] if pid_name else []))

        def _body(*args):
            operands = list(args)
            if pid_name is not None:
                operands.append(partition_id_tensor())
            outs = _bass_exec_p.bind(
                *operands,
                out_avals=tuple(out_avals),
                in_names=all_names,
                out_names=tuple(out_names),
                lowering_input_output_aliases=(),
                sim_require_finite=True,
                sim_require_nnan=True,
                nc=nc,
            )
            return tuple(outs)

        devices = jax.devices()[:n_cores]
        self.mesh = Mesh(np.asarray(devices), ("core",))
        self.sharding = NamedSharding(self.mesh, PartitionSpec("core"))
        n_out = len(out_names)
        donate = tuple(range(n_params, n_params + n_out))
        self.jitted = jax.jit(
            shard_map(_body, mesh=self.mesh,
                      in_specs=(PartitionSpec("core"),) * (n_params + n_out),
                      out_specs=(PartitionSpec("core"),) * n_out,
                      check_rep=False),
            donate_argnums=donate, keep_unused=True,
        )

    def stage_inputs(self, in_maps):
        per_core = [[np.asarray(m[name]) for name in self.in_names] for m in in_maps]
        concat = [np.concatenate([per_core[c][i] for c in range(self.n_cores)], axis=0)
                  for i in range(len(self.in_names))]
        return [jax.device_put(a, self.sharding) for a in concat]

    def stage_zeros(self):
        zs = [np.zeros((self.n_cores * z.shape[0], *z.shape[1:]), z.dtype)
              for z in self.zero_outs]
        return [jax.device_put(z, self.sharding) for z in zs]

    def run(self, dev_inputs, dev_zeros=None):
        if dev_zeros is None:
            dev_zeros = self.stage_zeros()
        outs = self.jitted(*dev_inputs, *dev_zeros)
        jax.block_until_ready(outs)
        return outs

    def unshard_out(self, outs):
        i = self.out_names.index("out")
        a = np.asarray(outs[i])  # (NCORES*NCHUNK, 128, 4096)
        a = a.reshape(NCORES, NCHUNK, 128, 4096)
        return np.concatenate([_decode_out(a[c]) for c in range(NCORES)], axis=0)


_RUNNER = None


def _get_runner():
    global _RUNNER
    if _RUNNER is None:
        nc = build_nc(B_LOCAL)
        _RUNNER = _SpmdRunner(nc, NCORES)
    return _RUNNER


def kernel(x1, x2, w, A, B, C):
    """Full-input entry point. Shards batch across 8 NeuronCores, runs the
    Bass kernel, gathers the full output (32768, 16, 64) float32."""
    runner = _get_runner()
    in_maps = prepare_in_maps(x1, x2, w, A, B, C)
    dev_in = runner.stage_inputs(in_maps)
    outs = runner.run(dev_in)
    return runner.unshard_out(outs)
